# revision 52
# baseline (speedup 1.0000x reference)
"""Trainium2 Bass kernel for nn_EnhancedDLinear (8-core SPMD, full I/O).

Mathematical reductions (verified against the jax reference):

1. ``LayerNorm(1)`` normalizes a size-1 axis, so its output is the constant
   ``ln_b``; the entire detail branch (convs, adaptive softmax, [N,S,S]
   attention) collapses to a host-precomputed constant row ``dp_row``.
2. The replicate-pad moving average (k=25) is a linear map folded into the
   first trend/seasonal MLP layers -> one [336, 336] weight ``w1``.
3. The channel-mean feeding the fusion MLP is computed by appending
   column-sum columns to the second-layer weights; per-output biases ride
   constant-1 rows (memset tiles / a tiny [1,*] bias tensor), zero
   broadcast DMAs.
4. The fusion-softmax weights stay UNNORMALIZED on chip; 1/den folds into
   the final Relu's per-partition scale, keeping the denominator broadcast
   and reciprocal off the critical path.

Sharding: batch b -> core b (B=8, 8 cores), zero collectives.

Precision: L1 runs in fp8-e4m3, weights pre-scaled x8 on host; the x8 is
carried through h1 (h1' = 8*h1, exact in bf16) and divided out of the
second-layer weights on the host, so L1 activations need no scale op and
split across ACT+DVE (DVE relu = tensor_scalar add,max). Validated
~1.6e-3 rel err vs the 2e-2 budget; everything accumulates in fp32.

DMA plan: x + all L1 weights pack into ONE fp8 tensor with 1296-byte
partition rows (sub-512B rows trigger the SDMA read-modify-write penalty),
split by columns across both HWDGE queues; per-queue bandwidth ~150 GB/s
gates the front, gpsimd/SWDGE (~1us issue) only carries late-needed data.
"""

import numpy as np
import ml_dtypes

import concourse.bacc as bacc
import concourse.tile as tile
from concourse import mybir
from concourse.bass_utils import run_bass_kernel_spmd

B, S, C, P = 8, 336, 96, 96
HID = 168
MAIN_K = 25
N_CORES = 8
KC = 112          # K chunk (336 = 3*112)

BF16 = ml_dtypes.bfloat16
FP8 = ml_dtypes.float8_e4m3
W1SCALE = 8.0
_CACHE = {}


def _mavg_matrix(s, k):
    # mt = xc @ Mm for the replicate-padded moving average
    p = (k - 1) // 2
    m = np.zeros((s, s), np.float64)
    for j in range(s):
        for d in range(-p, p + 1):
            i = min(max(j + d, 0), s - 1)
            m[i, j] += 1.0 / k
    return m.astype(np.float32)


def _build_module():
    f32 = mybir.dt.float32
    bf = mybir.dt.bfloat16
    f8 = mybir.dt.float8e4
    nc = bacc.Bacc("TRN2", target_bir_lowering=False, debug=False,
                   num_devices=N_CORES)

    # xw: [x chunks (288) | w1 chunk0 | chunk1 | chunk2] all fp8
    xw = nc.dram_tensor("xw", [KC, 1296], f8, kind="ExternalInput")
    w2p = nc.dram_tensor("w2p", [KC, 582], bf, kind="ExternalInput")
    w2b = nc.dram_tensor("w2b", [1, 582], bf, kind="ExternalInput")
    zpk = nc.dram_tensor("zpk", [96, 352], bf, kind="ExternalInput")
    opk = nc.dram_tensor("opk", [96, 240], bf, kind="ExternalInput")
    cf = nc.dram_tensor("cf", [KC, 8], f32, kind="ExternalInput")
    y = nc.dram_tensor("y", [P, P], f32, kind="ExternalOutput")

    AF = mybir.ActivationFunctionType
    OP = mybir.AluOpType

    with tile.TileContext(nc) as tc:
        with (
            tc.tile_pool(name="wp", bufs=1) as wp,
            tc.tile_pool(name="hp", bufs=1) as hp,
            tc.tile_pool(name="pp", bufs=8, space="PSUM") as pp,
        ):
            xws = wp.tile([KC, 1296], f8, tag="xws")
            w2s = wp.tile([KC, 582], bf, tag="w2s")
            w2bs = wp.tile([1, 582], bf, tag="w2bs")
            zps = wp.tile([96, 352], bf, tag="zps")
            ops = wp.tile([96, 240], bf, tag="ops")
            cfs = wp.tile([KC, 8], f32, tag="cfs")

            # constant-1 tiles for bias folding (memset; later ACT writes
            # overwrite rows 0:N, keeping the constant row)
            h1c = [hp.tile([KC, 96], bf, tag=f"h1c_{j}", name=f"h1c_{j}")
                   for j in range(3)]
            z1s = hp.tile([33, 1], bf, tag="z1s")
            hs = hp.tile([49, 96], bf, tag="hs")
            ones_bf = hp.tile([1, 96], bf, tag="ones_bf")
            nc.vector.memset(ones_bf, 1.0)   # first: gates the PE warmup
            nc.vector.memset(z1s, 1.0)
            nc.vector.memset(hs, 1.0)

            # HAM warmup: keep the PE busy during the input-DMA wait so the
            # clock gate opens (1.2 -> 2.4 GHz) before the real matmuls
            ps_w = pp.tile([96, 96], f32, tag="ps", name="ps_warm")
            for _ in range(23):
                nc.tensor.matmul(ps_w, ones_bf, ones_bf,
                                 start=True, stop=True)

            # need-ordered DMAs; L1 operands split across both HWDGE queues
            nc.sync.dma_start(out=xws[:, 0:648], in_=xw[:, 0:648])
            nc.scalar.dma_start(out=xws[:, 648:1296], in_=xw[:, 648:1296])
            nc.sync.dma_start(out=w2s[:, 0:288], in_=w2p[:, 0:288])
            nc.scalar.dma_start(out=w2s[:, 288:582], in_=w2p[:, 288:582])
            nc.gpsimd.dma_start(out=cfs, in_=cf[:, :])
            nc.sync.dma_start(out=zps, in_=zpk[:, :])
            nc.gpsimd.dma_start(out=w2bs, in_=w2b[:, :])
            nc.gpsimd.dma_start(out=ops, in_=opk[:, :])

            xbs = xws[:, 0:288]

            def wa(j, m):
                base = 288 + 336 * j + KC * m
                return xws[:, base:base + KC]

            # ---- L1: h1'[u, c] = relu((8*w1).T @ x + 8*b1) = 8*h1 ----
            # m-order 2,1,0 so the m=2 tile (which gates the ts2 chain via
            # its ACT) completes first; relus split ACT/DVE/ACT
            ps_l1 = {m: pp.tile([KC, 96], f32, tag="ps", name=f"ps_l1_{m}")
                     for m in range(3)}
            for j in range(3):
                for m in (2, 1, 0):
                    nc.tensor.matmul(
                        ps_l1[m], wa(j, m), xbs[:, 96 * j:96 * (j + 1)],
                        start=(j == 0), stop=(j == 2))
            # three relus on three different engines, fully parallel
            nc.scalar.activation(h1c[2], ps_l1[2], AF.Relu,
                                 bias=cfs[0:KC, 2:3])
            nc.vector.tensor_scalar(h1c[1], ps_l1[1], cfs[0:KC, 1:2], 0.0,
                                    OP.add, OP.max)
            nc.scalar.activation(h1c[0], ps_l1[0], AF.Relu,
                                 bias=cfs[0:KC, 0:1])

            # ---- ts2 (column sums) first: unblocks the softmax chain;
            # bias rides a constant-1-row matmul ----
            ps_ts = pp.tile([96, 2], f32, tag="ps", name="ps_ts")
            nc.tensor.matmul(ps_ts, ones_bf, w2bs[0:1, 576:578],
                             start=True, stop=False)
            for j in (2, 1, 0):
                nc.tensor.matmul(ps_ts, h1c[j], w2s[:, 576 + 2 * j:578 + 2 * j],
                                 start=False, stop=(j == 0))
            ts2b = hp.tile([96, 2], bf, tag="ts2b")
            nc.vector.tensor_copy(ts2b, ps_ts)

            # ---- fusion softmax chain (tp/sp matmuls are deliberately
            # AFTER z2 in program order: PE is strict FIFO, and the z-chain
            # gates the critical path while tp/sp aren't needed until the
            # G-combination — they fill PE idle time under the Exp) ----
            ps_z1 = pp.tile([32, 1], f32, tag="ps")
            nc.tensor.matmul(ps_z1, zps[:, 288:320], ts2b[:, 0:1],
                             start=True, stop=False)
            nc.tensor.matmul(ps_z1, zps[:, 320:352], ts2b[:, 1:2],
                             start=False, stop=True)
            nc.vector.tensor_scalar(z1s[0:32, :], ps_z1, cfs[0:32, 3:4], 0.0,
                                    OP.add, OP.max)

            ps_z2 = pp.tile([1, 288], f32, tag="ps")
            nc.tensor.matmul(ps_z2, z1s, zps[0:33, 0:288],
                             start=True, stop=True)

            # ---- L2 main: block-diagonal [tp | sp] ----
            ps_tpsp = pp.tile([96, 192], f32, tag="ps", name="ps_tpsp")
            nc.tensor.matmul(ps_tpsp, ones_bf, w2bs[0:1, 0:192],
                             start=True, stop=False)
            for j in (2, 1, 0):
                nc.tensor.matmul(ps_tpsp, h1c[j], w2s[:, 192 * j:192 * (j + 1)],
                                 start=False, stop=(j == 0))
            ps_tp = ps_tpsp[:, 0:96]
            ps_sp = ps_tpsp[:, 96:192]

            e_row = hp.tile([1, 288], f32, tag="e_row")
            den = hp.tile([1, 1], f32, tag="den")
            nc.scalar.activation(e_row, ps_z2, AF.Exp, accum_out=den)

            # transpose exp chunks into per-partition columns; weights stay
            # UNNORMALIZED — 1/den folds into the final Relu scale, so the
            # den-broadcast matmul + reciprocal run off the critical path
            onef = cfs[0:1, 5:6]
            ps_fw = pp.tile([96, 3], f32, tag="ps", name="ps_fw")
            for k in (2, 1, 0):   # k=2 first: gd below only needs column 2
                nc.tensor.matmul(ps_fw[:, k:k + 1],
                                 e_row[0:1, 96 * k:96 * (k + 1)], onef,
                                 is_transpose=True, skip_group_check=True)

            # den broadcast as a row (for the fp1b*den rank-1 bias below)
            den_row = hp.tile([1, 96], bf, tag="den_row")
            nc.vector.tensor_scalar_mul(den_row, ones_bf, den)
            # ... and down 48 partitions via a 1-instruction bf16 matmul
            # (bf16 den rounding cancels against the exact den_row bias term;
            # G is bf16 anyway)
            ps_dc = pp.tile([48, 1], f32, tag="ps", name="ps_dc")
            nc.tensor.matmul(ps_dc, ones_bf[0:1, 0:48], den_row[0:1, 0:1],
                             start=True, stop=True)

            # dp-component as a rank-1 update of ps_h: a1 = fp1wT.T @ fw_d',
            # then a1 (x) dp_row accumulates into the projection PSUM (ACT
            # copies + PE matmuls are idle-engine work off the DVE chain)
            fwd_sb = hp.tile([96, 1], bf, tag="fwd_sb")
            nc.scalar.activation(fwd_sb, ps_fw[:, 2:3], AF.Copy)
            ps_a = pp.tile([1, 48], f32, tag="ps", name="ps_a")
            nc.tensor.matmul(ps_a, fwd_sb, ops[:, 0:48],
                             start=True, stop=True)
            a1row = hp.tile([1, 48], bf, tag="a1row")
            nc.scalar.activation(a1row, ps_a, AF.Copy)

            # ---- G' = den*(fw_t*tp + fw_s*sp), c on partitions ----
            gs = hp.tile([96, 96], f32, tag="gs")
            nc.vector.tensor_scalar_mul(gs, ps_sp, ps_fw[:, 1:2])
            g = hp.tile([96, 96], bf, tag="g")
            nc.vector.scalar_tensor_tensor(g, ps_tp, ps_fw[:, 0:1], gs,
                                           OP.mult, OP.add)
            recip = hp.tile([48, 1], f32, tag="recip")
            nc.vector.reciprocal(recip, ps_dc)

            # ---- final projection; fp1b*den and the dp rank-1 term ride
            # extra matmuls so the relu is a 2-op DVE tensor_scalar ----
            ps_h = pp.tile([48, 96], f32, tag="ps")
            nc.tensor.matmul(ps_h, ops[:, 0:48], g, start=True, stop=False)
            nc.tensor.matmul(ps_h, a1row, ops[0:1, 48:144],
                             start=False, stop=False)
            nc.tensor.matmul(ps_h, w2bs[0:1, 192:240], den_row,
                             start=False, stop=True)
            nc.vector.tensor_scalar(hs[0:48, :], ps_h, recip, 0.0,
                                    OP.mult, OP.max)
            ps_o = pp.tile([96, 96], f32, tag="ps")
            nc.tensor.matmul(ps_o, hs, ops[0:49, 144:240],
                             start=True, stop=True)
            out_s = hp.tile([96, 96], f32, tag="out")
            nc.vector.tensor_copy(out_s, ps_o)
            nc.sync.dma_start(out=y[:, :], in_=out_s)

    nc.compile()
    return nc


def _prep_weights(i):
    f = np.float32
    mm = _mavg_matrix(S, MAIN_K)
    w1 = np.empty((S, 2 * HID), f)
    w1[:, :HID] = mm @ i['lt1w'].T.astype(f)
    w1[:, HID:] = (np.eye(S, dtype=f) - mm) @ i['ls1w'].T.astype(f)
    w1 *= W1SCALE

    # constant detail_pred row (LayerNorm(1) output == ln_b exactly)
    xf = np.full((S,), f(i['ln_b'][0]), f)
    dp_row = (np.maximum(xf @ i['op1w'].T + i['op1b'], 0)
              @ i['op2w'].T + i['op2b']).astype(f)
    b1f = (i['fn1b']
           + dp_row.mean(dtype=f) * i['fn1w'][:, 2 * C:].sum(1)).astype(f)

    lt2wt = np.ascontiguousarray(i['lt2w'].T, f) / W1SCALE   # [168, 96]
    ls2wt = np.ascontiguousarray(i['ls2w'].T, f) / W1SCALE
    lt2b = i['lt2b'].astype(f)
    ls2b = i['ls2b'].astype(f)
    lt2s = lt2wt.sum(1)
    ls2s = ls2wt.sum(1)

    # block-diagonal [tp | sp] (pre-divided by W1SCALE: h1' = 8*h1);
    # biases + bias sums live in w2b (constant-1 matmul row, unscaled)
    w2p = np.zeros((KC, 582), f)
    w2p[0:112, 0:96] = lt2wt[0:112]
    w2p[0:56, 192:288] = lt2wt[112:168]
    w2p[56:112, 288:384] = ls2wt[0:56]
    w2p[0:112, 480:576] = ls2wt[56:168]
    w2p[0:112, 576] = lt2s[0:112]
    w2p[0:56, 578] = lt2s[112:168]
    w2p[56:112, 579] = ls2s[0:56]
    w2p[0:112, 581] = ls2s[56:168]

    w2b = np.zeros((1, 582), f)
    w2b[0, 0:96] = lt2b
    w2b[0, 96:192] = ls2b
    w2b[0, 192:240] = i['fp1b']       # rank-1 bias row for ps_h
    w2b[0, 576] = lt2b.sum(dtype=f)
    w2b[0, 577] = ls2b.sum(dtype=f)

    zpk = np.zeros((96, 352), f)
    zpk[0:32, 0:288] = i['fn2w'].T
    zpk[32, 0:288] = i['fn2b']
    zpk[0:96, 288:320] = i['fn1w'][:, 0:C].T / C
    zpk[0:96, 320:352] = i['fn1w'][:, C:2 * C].T / C

    opk = np.zeros((96, 240), f)
    opk[0:96, 0:48] = i['fp1w'].T
    opk[0:96, 48:144] = np.tile(dp_row[None, :], (96, 1))
    opk[0:48, 144:240] = i['fp2w'].T
    opk[48, 144:240] = i['fp2b']

    cf = np.zeros((KC, 8), f)
    b1 = np.concatenate([i['lt1b'], i['ls1b']]).astype(f) * W1SCALE
    for j in range(3):
        cf[0:KC, j] = b1[KC * j:KC * (j + 1)]
    cf[0:32, 3] = b1f
    cf[0:48, 4] = i['fp1b']
    cf[0, 5] = 1.0

    wa8 = [np.ascontiguousarray(w1[KC * j:KC * (j + 1), :]).astype(FP8)
           for j in range(3)]
    return dict(w2p=w2p.astype(BF16), w2b=w2b.astype(BF16),
                zpk=zpk.astype(BF16), opk=opk.astype(BF16), cf=cf), wa8


def make_in_maps(inputs):
    shared, wa8 = _prep_weights(inputs)
    x = np.asarray(inputs['x'], np.float32)
    in_maps = []
    for b in range(N_CORES):
        xwp = np.empty((KC, 1296), FP8)
        for j in range(3):
            xwp[:, 96 * j:96 * (j + 1)] = x[b, KC * j:KC * (j + 1), :].astype(FP8)
            xwp[:, 288 + 336 * j:288 + 336 * (j + 1)] = wa8[j]
        in_maps.append(dict(shared, xw=xwp))
    return in_maps


def kernel(**inputs):
    if "nc" not in _CACHE:
        _CACHE["nc"] = _build_module()
    res = run_bass_kernel_spmd(_CACHE["nc"], make_in_maps(inputs),
                               core_ids=list(range(N_CORES)))
    return np.stack([res.results[b]["y"] for b in range(N_CORES)], 0)
